# revision 48
# baseline (speedup 1.0000x reference)
"""Trainium2 Bass kernel for nn_BilinearFusion.

out[b] = sum_h [ x1_h(b)·W1_h + b1_h + x2_h(b)·W2_h + x2_h(b)^T W3_h x1_h(b) ]

Host-side staging: shard batch across 8 cores; cast x1/x2 to bf16 and lay
them out pre-transposed per head (xt[i, h, b] = x[b, h*128+i]) in batch-major
1MB chunks so the device only does contiguous DMA loads.

t1 fold: for heads where c_h = W3_h^{-T} W1_h is small (|c|_inf <= CMAX),
replace x2_h by x2_h + c_h on the host.  Then
  (x2+c)^T W3 x1 = x2^T W3 x1 + W1·x1, and the W2-bias reduce picks up a
  constant W2·c (folded into the host-side scalar).  This removes the
  per-head t1 matmul from the PE for folded heads entirely.  Ill-conditioned
  heads (large c would blow up bf16 rounding) keep the explicit M=1 t1
  matmul.  Verified in emulation: rel err 0.0065 vs gate 0.02.

Device (per core, 2048 rows = 4 batches x 512), per batch (bats 0-2):
  PE  : yps_h = W3_h^T @ x1t_h            (8 streams, fp32 PSUM)
  PE  : rps[1,b] += W1_h^T @ x1t_h        (unfolded heads only)
  ACT : s_h = Identity(yps_h + W2col_h)   (folds t2; per-partition bias)
  DVE : prod_pair = s2 * x2t[2 heads]     (4 fused 2-head muls, 2x mode)
  DVE : halves = prods[0:4] + prods[4:8]  (one wide contiguous add)
  DVE : quads  = halves[0:2] + halves[2:4]
  PE  : rps += ones^T @ quads[0]; += ones^T @ quads[1]  (deferred 2 streams)
  ACT copies rps -> res slice; one sync DMA stores res.  Host adds consts.
Last batch: per-head muls + direct per-head PE reduces (lag-2 deferred) so
only one head's short chain trails the final DMA packet.

Measured engine busy (MID p-state): DMA ~24.5us (412GB/s sustained after
~3us spin-up), ACT ~22us, DVE ~19us, PE ~24us incl. warms.  The fixed
compiler epilogue (all-sem clear + barriers) adds ~9.2us inside the
measured window; nothing in-kernel can remove it.

Hard-won scheduling rules (measured on HW):
  - ONE clean HWDGE ring (sync) carries everything; a second ring or
    SWDGE traffic fragments packets and starves transfers.
  - Pool (gpsimd) slab ops are poison: ~1175ns each AND concurrent DVE
    ops slow ~3x (shared SBUF ports).  Keep Pool idle.
  - dma_start dispatches block their engine queue on flow-control sems, so
    the scalar/vector/tensor queues carry no DMA at all.
  - PE p-states: 634ns/512-col stream at MID (1.2GHz), 375ns at FULL
    (2.4GHz).  ~4us gapless matmul activity promotes; gaps <= ~1.2us can
    hold it, but a power/thermal limiter demotes unpredictably — design
    for MID, treat FULL as a bonus.
  - DMA engines re-ramp from ~5GB/s after any queue gap; a sacrificial
    warm transfer does NOT absorb the ramp (measured +8us — don't).
  - Run-to-run HW variance is +/-2.5us (thermal) — compare min-of-8.
"""

import numpy as np
import ml_dtypes

import concourse.bass as bass
import concourse.tile as tile
from concourse import bacc, mybir
from concourse.bass_utils import run_bass_kernel_spmd

BF16 = ml_dtypes.bfloat16

B, D, HEAD, DIM = 16384, 1024, 8, 128
NCORES = 8
ROWS = B // NCORES          # 2048 rows per core
P = 128
BATCH = 512                 # rows per batch (moving free dim of matmuls)
NB = ROWS // BATCH          # 4 batches

CMAX = 8.0                  # |c|_inf gate for the t1 fold
N_WARM = 6                  # PE clock-gate warm-up matmuls: sized so the
                            # warm burst runs seamlessly into batch 0's
                            # matmuls — ~4us of gapless activity promotes
                            # the clock MID->FULL (634->379ns streams);
                            # the p-state is also power/thermal limited,
                            # so promotion is best-effort

_nc_cache = []              # [0]: nc, [1]: unfolded tuple it was built for


def build_nc(unfolded=(1,)):
    nc = bacc.Bacc(target_bir_lowering=False)
    f32 = mybir.dt.float32
    bf16 = mybir.dt.bfloat16
    Act = mybir.ActivationFunctionType

    x1t_d = nc.dram_tensor("x1t", [NB, P, HEAD, BATCH], bf16,
                           kind="ExternalInput")
    x2t_d = nc.dram_tensor("x2t", [NB, P, HEAD, BATCH], bf16,
                           kind="ExternalInput")
    w3t_d = nc.dram_tensor("w3t", [DIM, HEAD, DIM], bf16, kind="ExternalInput")
    w12f_d = nc.dram_tensor("w12f", [DIM, 2, HEAD], f32, kind="ExternalInput")
    out_d = nc.dram_tensor("out", [NB * BATCH], f32, kind="ExternalOutput")

    with tile.TileContext(nc) as tc:
        with (
            tc.tile_pool(name="const", bufs=1) as const_pool,
            tc.tile_pool(name="xt", bufs=NB) as xt_pool,
            tc.tile_pool(name="s", bufs=8) as s_pool,
            tc.tile_pool(name="prod", bufs=3) as prod_pool,
            tc.tile_pool(name="acc", bufs=2) as acc_pool,
            tc.tile_pool(name="res", bufs=1) as res_pool,
            tc.tile_pool(name="yps", bufs=5, space="PSUM") as yps_pool,
            tc.tile_pool(name="rps", bufs=2, space="PSUM") as rps_pool,
        ):
            ones = const_pool.tile([DIM, 1], bf16)
            nc.vector.memset(ones, 1.0)

            # weights ride the FRONT of the sync ring (a separate ring gets
            # starved by the bulk stream; measured 10us for 275KB).
            w12b = const_pool.tile([DIM, 2, HEAD], bf16)
            w12f = const_pool.tile([DIM, 2, HEAD], f32)
            w3l = const_pool.tile([DIM, HEAD, DIM], bf16)


            x1ts = [xt_pool.tile([P, HEAD, BATCH], bf16, tag="x1t",
                                 name=f"x1s{b}") for b in range(NB)]
            x2ts = [xt_pool.tile([P, HEAD, BATCH], bf16, tag="x2t",
                                 name=f"x2s{b}") for b in range(NB)]
            def ld(ts, td, bat, a, b):
                nc.sync.dma_start(out=ts[bat][:, a:b, :],
                                  in_=td[bat, :, a:b, :])

            # x1 rides one FULL batch ahead of x2 (PE consumes only x1):
            # x1(b+1) halves are woven between x2(b) halves, so the PE/ACT
            # chain for batch b+1 is already hot when the DVE finishes
            # batch b — kills the ~2us ACT-wait stall at each batch
            # boundary.  Ring head: x1 front absorbs the DMA spin-up
            # (first packets crawl at ~5GB/s/engine), then w3l/w12f ride
            # hot.  Last batch's x2 tail arrives in eighth-chunks; the
            # dispatches are all queued early so each tail chunk only pays
            # its own transfer+sem, and only one head trails the final
            # packet.
            ld(x1ts, x1t_d, 0, 0, 4)
            nc.sync.dma_start(out=w3l, in_=w3t_d[:])
            nc.sync.dma_start(out=w12f, in_=w12f_d[:])
            nc.vector.tensor_copy(w12b, w12f)
            ld(x2ts, x2t_d, 0, 0, 2)
            ld(x1ts, x1t_d, 0, 4, 8)
            ld(x2ts, x2t_d, 0, 2, 4)
            for bat in range(1, NB):
                ld(x1ts, x1t_d, bat, 0, 4)
                if bat == NB - 1:
                    # tail batch's x1 back half rides 1.5 batches ahead:
                    # at MID clock s(h7) <- ACT <- W3 <- x1[4:8] lands
                    # only ~0.1us before the last x2 chunk — this buys the
                    # tail s-chain 0.64us of margin, delaying only b2's
                    # non-critical finalize.
                    ld(x1ts, x1t_d, bat, 4, 8)
                    ld(x2ts, x2t_d, bat - 1, 4, 8)
                else:
                    ld(x2ts, x2t_d, bat - 1, 4, 8)
                    ld(x1ts, x1t_d, bat, 4, 8)
                ld(x2ts, x2t_d, bat, 0, 4)
            ld(x2ts, x2t_d, NB - 1, 4, 6)
            ld(x2ts, x2t_d, NB - 1, 6, 8)

            # warm the ACT function table while DMA streams
            warm = const_pool.tile([1, 1], f32)
            nc.scalar.activation(warm, ones[0:1, 0:1], Act.Identity,
                                 bias=0.0, scale=1.0)

            # warm the PE HAM clock-gate during the DMA fill
            scratch = const_pool.tile([DIM, BATCH], bf16)
            nc.vector.memset(scratch, 0.0)
            wps = yps_pool.tile([DIM, BATCH], f32, tag="warm", bufs=1)
            for _ in range(N_WARM):
                nc.tensor.matmul(wps, scratch[:, 0:DIM], scratch,
                                 start=True, stop=True)

            res = res_pool.tile([1, NB * BATCH], f32)

            # deferred thunks so the PE never waits on the DVE round
            # trip: finalizers (prev batch reduce+copy+store) drain at h==5
            # of the next batch; the last batch's tail reduces use their own
            # lagged list.
            finalizers = []
            tail = []

            def drain(lst, n):
                while len(lst) > n:
                    lst.pop(0)()

            for bat in range(NB):
                last = bat == NB - 1
                x1t, x2t = x1ts[bat], x2ts[bat]
                rps = rps_pool.tile([1, BATCH], f32)
                prods = prod_pool.tile([DIM, HEAD, BATCH], bf16, tag="p")
                pairs = prod_pool.tile([DIM, HEAD // 2, BATCH], bf16,
                                       tag="q")
                quads = acc_pool.tile([DIM, 2, BATCH], bf16, tag="a")
                state = {"open": False}

                def touch_rps(rps=rps, state=state):
                    st = not state["open"]
                    state["open"] = True
                    return st

                def finalize(rps=rps, quads=quads, bat=bat, state=state):
                    nc.tensor.matmul(rps, ones, quads[:, 0, :],
                                     start=not state["open"], stop=False)
                    nc.tensor.matmul(rps, ones, quads[:, 1, :],
                                     start=False, stop=True)
                    nc.scalar.copy(res[:, bat * BATCH:(bat + 1) * BATCH],
                                   rps)
                    nc.sync.dma_start(
                        out=out_d[bat * BATCH:(bat + 1) * BATCH],
                        in_=res[:, bat * BATCH:(bat + 1) * BATCH])

                s2 = None
                for h in range(HEAD):
                    yps = yps_pool.tile([DIM, BATCH], f32)
                    nc.tensor.matmul(yps, w3l[:, h, :], x1t[:, h, :],
                                     start=True, stop=True)
                    if h in unfolded:
                        nc.tensor.matmul(rps, w12b[:, 0, h:h + 1],
                                         x1t[:, h, :],
                                         start=touch_rps(), stop=False)
                    if h % 2 == 0:
                        s2 = s_pool.tile([DIM, 2, BATCH], bf16, tag="s")
                    nc.scalar.activation(s2[:, h % 2, :], yps, Act.Identity,
                                         bias=w12f[:, 1, h:h + 1],
                                         scale=1.0)
                    if h == 7:
                        # drain AFTER all of this batch's W3 matmuls AND
                        # after the h7 activation: at h==5 the prev batch's
                        # reduce streams displace W3 h6/h7 by ~1.2us, and
                        # draining before the ACT emission would park the
                        # prev batch's res-copy ahead of s(h7) in the
                        # in-order ACT queue — blocking the tail-critical
                        # s on the last batch.
                        drain(finalizers, 0)
                    if not last:
                        # 2-head fused mul + tree sum as wide contiguous
                        # DVE ops — one op amortizes the ~175ns fixed
                        # overhead over 1024/2048 elements.  Pool is
                        # avoided entirely: its slab ops measured 1175ns
                        # AND slow concurrent DVE ops ~3x (shared SBUF
                        # ports).
                        if h % 2 == 1:
                            nc.vector.tensor_mul(prods[:, h - 1:h + 1, :],
                                                 s2, x2t[:, h - 1:h + 1, :])
                        if h == 7:
                            nc.vector.tensor_add(pairs,
                                                 prods[:, 0:4, :],
                                                 prods[:, 4:8, :])
                            nc.vector.tensor_add(quads,
                                                 pairs[:, 0:2, :],
                                                 pairs[:, 2:4, :])
                    else:
                        # tail batch: per-head muls and direct per-head PE
                        # reduces (DMA is done, the PE is otherwise idle) —
                        # shortest chain after the last packet.
                        prod = prods[:, h, :]
                        nc.vector.tensor_mul(prod, s2[:, h % 2, :],
                                             x2t[:, h, :])
                        def red(rps=rps, prod=prod, h=h, bat=bat):
                            nc.tensor.matmul(rps, ones, prod,
                                             start=touch_rps(),
                                             stop=(h == HEAD - 1))
                            if h == HEAD - 1:
                                nc.scalar.copy(
                                    res[:, bat * BATCH:(bat + 1) * BATCH],
                                    rps)
                                nc.sync.dma_start(
                                    out=out_d[bat * BATCH:
                                              (bat + 1) * BATCH],
                                    in_=res[:, bat * BATCH:
                                            (bat + 1) * BATCH])
                        # do NOT drain mid-loop: a red waits its x2-gated
                        # prod, and parking it in the in-order PE queue
                        # before W3(h+1..) blocks the x1-ready matmuls —
                        # delaying the tail-critical s(h6,h7) chain.  All
                        # reds drain after the loop, behind W3(h7)/s(h7).
                        tail.append(red)
                if not last:
                    finalizers.append(finalize)
                else:
                    # 4 scratch matmuls fill the ~2.4us data-gated PE idle
                    # between the b2 finalize reduces and the x2-gated tail
                    # reduces: long idles here demote the clock and the
                    # whole 8-reduce tail train then runs 634 vs 375ns.
                    # They execute exactly in the gap (queued after the
                    # finalize drain, before the tail reds).
                    for _ in range(4):
                        nc.tensor.matmul(wps, scratch[:, 0:DIM], scratch,
                                         start=True, stop=True)
                    drain(tail, 0)

    nc.finalize()
    return nc


def _fold(W1, W2, W3):
    """Per-head c_h with |c|_inf <= CMAX s.t. W3_h^T c_h = W1_h (t1 fold).

    Returns (cs [HEAD, DIM] float64, unfolded tuple, const): heads whose
    solve is too ill-conditioned keep c=0 and an explicit t1 matmul.
    """
    W1 = np.asarray(W1, np.float64)
    W2 = np.asarray(W2, np.float64)
    w3b = np.asarray(W3, np.float32).astype(BF16).astype(np.float64)
    cs = np.zeros((HEAD, DIM))
    unfolded = []
    for h in range(HEAD):
        try:
            c = np.linalg.solve(w3b[h].T, W1[h])
        except np.linalg.LinAlgError:
            c = np.full(DIM, np.inf)
        if np.abs(c).max() <= CMAX and np.all(np.isfinite(c)):
            cs[h] = c
        else:
            unfolded.append(h)
    const = -(W2 * cs).sum()
    return cs, tuple(unfolded), const


def _prep_weights(W1, W2, W3):
    # W3 is [h, o, i]; lhsT needs [i (partitions), h, o]
    w3t = np.ascontiguousarray(
        np.transpose(np.asarray(W3), (2, 0, 1))).astype(BF16)
    w12f = np.empty((DIM, 2, HEAD), dtype=np.float32)
    w12f[:, 0, :] = np.asarray(W1).T   # [i, h]
    w12f[:, 1, :] = np.asarray(W2).T   # [o, h]
    return w3t, w12f


def _prep_x(x, cs=None):
    """[B, D] fp32 -> per-core [NB, P, HEAD, BATCH] bf16, pre-transposed.
    cs: optional [HEAD, DIM] fold offsets added before the bf16 cast."""
    xv = np.asarray(x, dtype=np.float64).reshape(NCORES, NB, BATCH, HEAD, DIM)
    if cs is not None:
        xv = xv + cs[None, None, None]
    xb = xv.astype(BF16)
    # [core, bat, b, h, i] -> [core, bat, i, h, b]
    v = xb.transpose(0, 1, 4, 3, 2)
    return np.ascontiguousarray(v)


def _in_maps(x1, x2, W1, W2, W3):
    cs, unfolded, const = _fold(W1, W2, W3)
    w3t, w12f = _prep_weights(W1, W2, W3)
    x1t = _prep_x(x1)
    x2t = _prep_x(x2, cs)
    maps = [
        {"x1t": x1t[c], "x2t": x2t[c], "w3t": w3t, "w12f": w12f}
        for c in range(NCORES)
    ]
    return maps, unfolded, const


def kernel(x1, x2, W1, b1, W2, W3):
    in_maps, unfolded, const = _in_maps(x1, x2, W1, W2, W3)
    if not _nc_cache or _nc_cache[1] != unfolded:
        _nc_cache.clear()
        _nc_cache.append(build_nc(unfolded))
        _nc_cache.append(unfolded)
    nc = _nc_cache[0]

    c_all = float(np.asarray(b1, dtype=np.float64).sum() + const)

    res = run_bass_kernel_spmd(nc, in_maps, core_ids=list(range(NCORES)))
    out = np.concatenate(
        [res.results[c]["out"].reshape(-1) for c in range(NCORES)])
    return (out + np.float32(c_all)).astype(np.float32)


# revision 50
# speedup vs baseline: 1.0095x; 1.0095x over previous
"""Trainium2 Bass kernel for nn_BilinearFusion.

out[b] = sum_h [ x1_h(b)·W1_h + b1_h + x2_h(b)·W2_h + x2_h(b)^T W3_h x1_h(b) ]

Host-side staging: shard batch across 8 cores; cast x1/x2 to bf16 and lay
them out pre-transposed per head (xt[i, h, b] = x[b, h*128+i]) in batch-major
1MB chunks so the device only does contiguous DMA loads.

t1 fold: for heads where c_h = W3_h^{-T} W1_h is small (|c|_inf <= CMAX),
replace x2_h by x2_h + c_h on the host.  Then
  (x2+c)^T W3 x1 = x2^T W3 x1 + W1·x1, and the W2-bias reduce picks up a
  constant W2·c (folded into the host-side scalar).  This removes the
  per-head t1 matmul from the PE for folded heads entirely.  Ill-conditioned
  heads (large c would blow up bf16 rounding) keep the explicit M=1 t1
  matmul.  Verified in emulation: rel err 0.0065 vs gate 0.02.

Device (per core, 2048 rows = 4 batches x 512), per batch (bats 0-2):
  PE  : yps_h = W3_h^T @ x1t_h            (8 streams, fp32 PSUM)
  PE  : rps[1,b] += W1_h^T @ x1t_h        (unfolded heads only)
  ACT : s_h = Identity(yps_h + W2col_h)   (folds t2; per-partition bias)
  DVE : prod_pair = s2 * x2t[2 heads]     (4 fused 2-head muls, 2x mode)
  DVE : halves = prods[0:4] + prods[4:8]  (one wide contiguous add)
  DVE : quads  = halves[0:2] + halves[2:4]
  PE  : rps += ones^T @ quads[0]; += ones^T @ quads[1]  (deferred 2 streams)
  ACT copies rps -> res slice; one sync DMA stores res.  Host adds consts.
Last batch: per-head muls + direct per-head PE reduces (lag-2 deferred) so
only one head's short chain trails the final DMA packet.

Measured engine busy (MID p-state): DMA ~24.5us (412GB/s sustained after
~3us spin-up), ACT ~22us, DVE ~19us, PE ~24us incl. warms.  The fixed
compiler epilogue (all-sem clear + barriers) adds ~9.2us inside the
measured window; nothing in-kernel can remove it.

Hard-won scheduling rules (measured on HW):
  - ONE clean HWDGE ring (sync) carries everything; a second ring or
    SWDGE traffic fragments packets and starves transfers.
  - Pool (gpsimd) slab ops are poison: ~1175ns each AND concurrent DVE
    ops slow ~3x (shared SBUF ports).  Keep Pool idle.
  - dma_start dispatches block their engine queue on flow-control sems, so
    the scalar/vector/tensor queues carry no DMA at all.
  - PE p-states: 634ns/512-col stream at MID (1.2GHz), 375ns at FULL
    (2.4GHz).  ~4us gapless matmul activity promotes; gaps <= ~1.2us can
    hold it, but a power/thermal limiter demotes unpredictably — design
    for MID, treat FULL as a bonus.
  - DMA engines re-ramp from ~5GB/s after any queue gap; a sacrificial
    warm transfer does NOT absorb the ramp (measured +8us — don't).
  - Run-to-run HW variance is +/-2.5us (thermal) — compare min-of-8.
"""

import numpy as np
import ml_dtypes

import concourse.bass as bass
import concourse.tile as tile
from concourse import bacc, mybir
from concourse.bass_utils import run_bass_kernel_spmd

BF16 = ml_dtypes.bfloat16

B, D, HEAD, DIM = 16384, 1024, 8, 128
NCORES = 8
ROWS = B // NCORES          # 2048 rows per core
P = 128
BATCH = 512                 # rows per batch (moving free dim of matmuls)
NB = ROWS // BATCH          # 4 batches

CMAX = 8.0                  # |c|_inf gate for the t1 fold
N_WARM = 0                  # PE clock-gate warm-up matmuls: sized so the
                            # warm burst runs seamlessly into batch 0's
                            # matmuls — ~4us of gapless activity promotes
                            # the clock MID->FULL (634->379ns streams);
                            # the p-state is also power/thermal limited,
                            # so promotion is best-effort

_nc_cache = []              # [0]: nc, [1]: unfolded tuple it was built for


def build_nc(unfolded=(1,)):
    nc = bacc.Bacc(target_bir_lowering=False)
    f32 = mybir.dt.float32
    bf16 = mybir.dt.bfloat16
    Act = mybir.ActivationFunctionType

    x1t_d = nc.dram_tensor("x1t", [NB, P, HEAD, BATCH], bf16,
                           kind="ExternalInput")
    x2t_d = nc.dram_tensor("x2t", [NB, P, HEAD, BATCH], bf16,
                           kind="ExternalInput")
    w3t_d = nc.dram_tensor("w3t", [DIM, HEAD, DIM], bf16, kind="ExternalInput")
    w12f_d = nc.dram_tensor("w12f", [DIM, 2, HEAD], f32, kind="ExternalInput")
    out_d = nc.dram_tensor("out", [NB * BATCH], f32, kind="ExternalOutput")

    with tile.TileContext(nc) as tc:
        with (
            tc.tile_pool(name="const", bufs=1) as const_pool,
            tc.tile_pool(name="xt", bufs=NB) as xt_pool,
            tc.tile_pool(name="s", bufs=8) as s_pool,
            tc.tile_pool(name="prod", bufs=3) as prod_pool,
            tc.tile_pool(name="acc", bufs=2) as acc_pool,
            tc.tile_pool(name="res", bufs=1) as res_pool,
            tc.tile_pool(name="yps", bufs=5, space="PSUM") as yps_pool,
            tc.tile_pool(name="rps", bufs=2, space="PSUM") as rps_pool,
        ):
            ones = const_pool.tile([DIM, 1], bf16)
            nc.vector.memset(ones, 1.0)

            # weights ride the FRONT of the sync ring (a separate ring gets
            # starved by the bulk stream; measured 10us for 275KB).
            w12b = const_pool.tile([DIM, 2, HEAD], bf16)
            w12f = const_pool.tile([DIM, 2, HEAD], f32)
            w3l = const_pool.tile([DIM, HEAD, DIM], bf16)


            x1ts = [xt_pool.tile([P, HEAD, BATCH], bf16, tag="x1t",
                                 name=f"x1s{b}") for b in range(NB)]
            x2ts = [xt_pool.tile([P, HEAD, BATCH], bf16, tag="x2t",
                                 name=f"x2s{b}") for b in range(NB)]
            def ld(ts, td, bat, a, b):
                nc.sync.dma_start(out=ts[bat][:, a:b, :],
                                  in_=td[bat, :, a:b, :])

            # x1 rides one FULL batch ahead of x2 (PE consumes only x1):
            # x1(b+1) halves are woven between x2(b) halves, so the PE/ACT
            # chain for batch b+1 is already hot when the DVE finishes
            # batch b — kills the ~2us ACT-wait stall at each batch
            # boundary.  Ring head: x1 front absorbs the DMA spin-up
            # (first packets crawl at ~5GB/s/engine), then w3l/w12f ride
            # hot.  Last batch's x2 tail arrives in eighth-chunks; the
            # dispatches are all queued early so each tail chunk only pays
            # its own transfer+sem, and only one head trails the final
            # packet.
            ld(x1ts, x1t_d, 0, 0, 4)
            nc.sync.dma_start(out=w3l, in_=w3t_d[:])
            nc.sync.dma_start(out=w12f, in_=w12f_d[:])
            nc.vector.tensor_copy(w12b, w12f)
            ld(x2ts, x2t_d, 0, 0, 2)
            ld(x1ts, x1t_d, 0, 4, 8)
            ld(x2ts, x2t_d, 0, 2, 4)
            for bat in range(1, NB):
                ld(x1ts, x1t_d, bat, 0, 4)
                if bat == NB - 1:
                    # tail batch's x1 back half rides 1.5 batches ahead:
                    # at MID clock s(h7) <- ACT <- W3 <- x1[4:8] lands
                    # only ~0.1us before the last x2 chunk — this buys the
                    # tail s-chain 0.64us of margin, delaying only b2's
                    # non-critical finalize.
                    ld(x1ts, x1t_d, bat, 4, 8)
                    ld(x2ts, x2t_d, bat - 1, 4, 8)
                else:
                    ld(x2ts, x2t_d, bat - 1, 4, 8)
                    ld(x1ts, x1t_d, bat, 4, 8)
                ld(x2ts, x2t_d, bat, 0, 4)
            ld(x2ts, x2t_d, NB - 1, 4, 6)
            ld(x2ts, x2t_d, NB - 1, 6, 8)

            # warm the ACT function table while DMA streams
            warm = const_pool.tile([1, 1], f32)
            nc.scalar.activation(warm, ones[0:1, 0:1], Act.Identity,
                                 bias=0.0, scale=1.0)

            # warm the PE HAM clock-gate during the DMA fill
            scratch = const_pool.tile([DIM, BATCH], bf16)
            nc.vector.memset(scratch, 0.0)
            wps = yps_pool.tile([DIM, BATCH], f32, tag="warm", bufs=1)
            for _ in range(N_WARM):
                nc.tensor.matmul(wps, scratch[:, 0:DIM], scratch,
                                 start=True, stop=True)

            res = res_pool.tile([1, NB * BATCH], f32)

            # deferred thunks so the PE never waits on the DVE round
            # trip: finalizers (prev batch reduce+copy+store) drain at h==5
            # of the next batch; the last batch's tail reduces use their own
            # lagged list.
            finalizers = []
            tail = []

            def drain(lst, n):
                while len(lst) > n:
                    lst.pop(0)()

            for bat in range(NB):
                last = bat == NB - 1
                x1t, x2t = x1ts[bat], x2ts[bat]
                rps = rps_pool.tile([1, BATCH], f32)
                prods = prod_pool.tile([DIM, HEAD, BATCH], bf16, tag="p")
                pairs = prod_pool.tile([DIM, HEAD // 2, BATCH], bf16,
                                       tag="q")
                quads = acc_pool.tile([DIM, 2, BATCH], bf16, tag="a")
                state = {"open": False}

                def touch_rps(rps=rps, state=state):
                    st = not state["open"]
                    state["open"] = True
                    return st

                def finalize(rps=rps, quads=quads, bat=bat, state=state):
                    nc.tensor.matmul(rps, ones, quads[:, 0, :],
                                     start=not state["open"], stop=False)
                    nc.tensor.matmul(rps, ones, quads[:, 1, :],
                                     start=False, stop=True)
                    nc.scalar.copy(res[:, bat * BATCH:(bat + 1) * BATCH],
                                   rps)
                    nc.sync.dma_start(
                        out=out_d[bat * BATCH:(bat + 1) * BATCH],
                        in_=res[:, bat * BATCH:(bat + 1) * BATCH])

                s2 = None
                for h in range(HEAD):
                    yps = yps_pool.tile([DIM, BATCH], f32)
                    nc.tensor.matmul(yps, w3l[:, h, :], x1t[:, h, :],
                                     start=True, stop=True)
                    if h in unfolded:
                        nc.tensor.matmul(rps, w12b[:, 0, h:h + 1],
                                         x1t[:, h, :],
                                         start=touch_rps(), stop=False)
                    if h % 2 == 0:
                        s2 = s_pool.tile([DIM, 2, BATCH], bf16, tag="s")
                    nc.scalar.activation(s2[:, h % 2, :], yps, Act.Identity,
                                         bias=w12f[:, 1, h:h + 1],
                                         scale=1.0)
                    if h == 7:
                        # drain AFTER all of this batch's W3 matmuls AND
                        # after the h7 activation: at h==5 the prev batch's
                        # reduce streams displace W3 h6/h7 by ~1.2us, and
                        # draining before the ACT emission would park the
                        # prev batch's res-copy ahead of s(h7) in the
                        # in-order ACT queue — blocking the tail-critical
                        # s on the last batch.
                        drain(finalizers, 0)
                    if not last:
                        # 2-head fused mul + tree sum as wide contiguous
                        # DVE ops — one op amortizes the ~175ns fixed
                        # overhead over 1024/2048 elements.  Pool is
                        # avoided entirely: its slab ops measured 1175ns
                        # AND slow concurrent DVE ops ~3x (shared SBUF
                        # ports).
                        if h % 2 == 1:
                            nc.vector.tensor_mul(prods[:, h - 1:h + 1, :],
                                                 s2, x2t[:, h - 1:h + 1, :])
                        if h == 7:
                            nc.vector.tensor_add(pairs,
                                                 prods[:, 0:4, :],
                                                 prods[:, 4:8, :])
                            nc.vector.tensor_add(quads,
                                                 pairs[:, 0:2, :],
                                                 pairs[:, 2:4, :])
                    else:
                        # tail batch: per-head muls and direct per-head PE
                        # reduces (DMA is done, the PE is otherwise idle) —
                        # shortest chain after the last packet.
                        prod = prods[:, h, :]
                        nc.vector.tensor_mul(prod, s2[:, h % 2, :],
                                             x2t[:, h, :])
                        def red(rps=rps, prod=prod, h=h, bat=bat):
                            nc.tensor.matmul(rps, ones, prod,
                                             start=touch_rps(),
                                             stop=(h == HEAD - 1))
                            if h == HEAD - 1:
                                nc.scalar.copy(
                                    res[:, bat * BATCH:(bat + 1) * BATCH],
                                    rps)
                                nc.sync.dma_start(
                                    out=out_d[bat * BATCH:
                                              (bat + 1) * BATCH],
                                    in_=res[:, bat * BATCH:
                                            (bat + 1) * BATCH])
                        # do NOT drain mid-loop: a red waits its x2-gated
                        # prod, and parking it in the in-order PE queue
                        # before W3(h+1..) blocks the x1-ready matmuls —
                        # delaying the tail-critical s(h6,h7) chain.  All
                        # reds drain after the loop, behind W3(h7)/s(h7).
                        tail.append(red)
                if not last:
                    finalizers.append(finalize)
                else:
                    drain(tail, 0)

    nc.finalize()
    return nc


def _fold(W1, W2, W3):
    """Per-head c_h with |c|_inf <= CMAX s.t. W3_h^T c_h = W1_h (t1 fold).

    Returns (cs [HEAD, DIM] float64, unfolded tuple, const): heads whose
    solve is too ill-conditioned keep c=0 and an explicit t1 matmul.
    """
    W1 = np.asarray(W1, np.float64)
    W2 = np.asarray(W2, np.float64)
    w3b = np.asarray(W3, np.float32).astype(BF16).astype(np.float64)
    cs = np.zeros((HEAD, DIM))
    unfolded = []
    for h in range(HEAD):
        try:
            c = np.linalg.solve(w3b[h].T, W1[h])
        except np.linalg.LinAlgError:
            c = np.full(DIM, np.inf)
        if np.abs(c).max() <= CMAX and np.all(np.isfinite(c)):
            cs[h] = c
        else:
            unfolded.append(h)
    const = -(W2 * cs).sum()
    return cs, tuple(unfolded), const


def _prep_weights(W1, W2, W3):
    # W3 is [h, o, i]; lhsT needs [i (partitions), h, o]
    w3t = np.ascontiguousarray(
        np.transpose(np.asarray(W3), (2, 0, 1))).astype(BF16)
    w12f = np.empty((DIM, 2, HEAD), dtype=np.float32)
    w12f[:, 0, :] = np.asarray(W1).T   # [i, h]
    w12f[:, 1, :] = np.asarray(W2).T   # [o, h]
    return w3t, w12f


def _prep_x(x, cs=None):
    """[B, D] fp32 -> per-core [NB, P, HEAD, BATCH] bf16, pre-transposed.
    cs: optional [HEAD, DIM] fold offsets added before the bf16 cast."""
    xv = np.asarray(x, dtype=np.float64).reshape(NCORES, NB, BATCH, HEAD, DIM)
    if cs is not None:
        xv = xv + cs[None, None, None]
    xb = xv.astype(BF16)
    # [core, bat, b, h, i] -> [core, bat, i, h, b]
    v = xb.transpose(0, 1, 4, 3, 2)
    return np.ascontiguousarray(v)


def _in_maps(x1, x2, W1, W2, W3):
    cs, unfolded, const = _fold(W1, W2, W3)
    w3t, w12f = _prep_weights(W1, W2, W3)
    x1t = _prep_x(x1)
    x2t = _prep_x(x2, cs)
    maps = [
        {"x1t": x1t[c], "x2t": x2t[c], "w3t": w3t, "w12f": w12f}
        for c in range(NCORES)
    ]
    return maps, unfolded, const


def kernel(x1, x2, W1, b1, W2, W3):
    in_maps, unfolded, const = _in_maps(x1, x2, W1, W2, W3)
    if not _nc_cache or _nc_cache[1] != unfolded:
        _nc_cache.clear()
        _nc_cache.append(build_nc(unfolded))
        _nc_cache.append(unfolded)
    nc = _nc_cache[0]

    c_all = float(np.asarray(b1, dtype=np.float64).sum() + const)

    res = run_bass_kernel_spmd(nc, in_maps, core_ids=list(range(NCORES)))
    out = np.concatenate(
        [res.results[c]["out"].reshape(-1) for c in range(NCORES)])
    return (out + np.float32(c_all)).astype(np.float32)


# revision 52
# speedup vs baseline: 1.0096x; 1.0001x over previous
"""Trainium2 Bass kernel for nn_BilinearFusion.

out[b] = sum_h [ x1_h(b)·W1_h + b1_h + x2_h(b)·W2_h + x2_h(b)^T W3_h x1_h(b) ]

Host-side staging: shard batch across 8 cores; cast x1/x2 to bf16 and lay
them out pre-transposed per head (xt[i, h, b] = x[b, h*128+i]) in batch-major
1MB chunks so the device only does contiguous DMA loads.

t1 fold: for heads where c_h = W3_h^{-T} W1_h is small (|c|_inf <= CMAX),
replace x2_h by x2_h + c_h on the host.  Then
  (x2+c)^T W3 x1 = x2^T W3 x1 + W1·x1, and the W2-bias reduce picks up a
  constant W2·c (folded into the host-side scalar).  This removes the
  per-head t1 matmul from the PE for folded heads entirely.  Ill-conditioned
  heads (large c would blow up bf16 rounding) keep the explicit M=1 t1
  matmul.  Verified in emulation: rel err 0.0065 vs gate 0.02.

Device (per core, 2048 rows = 4 batches x 512), per batch (bats 0-2):
  PE  : yps_h = W3_h^T @ x1t_h            (8 streams, fp32 PSUM)
  PE  : rps[1,b] += W1_h^T @ x1t_h        (unfolded heads only)
  ACT : s_h = Identity(yps_h + W2col_h)   (folds t2; per-partition bias)
  DVE : prod_pair = s2 * x2t[2 heads]     (4 fused 2-head muls, 2x mode)
  DVE : halves = prods[0:4] + prods[4:8]  (one wide contiguous add)
  DVE : quads  = halves[0:2] + halves[2:4]
  PE  : rps += ones^T @ quads[0]; += ones^T @ quads[1]  (deferred 2 streams)
  ACT copies rps -> res slice; one sync DMA stores res.  Host adds consts.
Last batch: per-head muls + direct per-head PE reduces (lag-2 deferred) so
only one head's short chain trails the final DMA packet.

Measured engine busy (MID p-state): DMA ~24.5us (412GB/s sustained after
~3us spin-up), ACT ~22us, DVE ~19us, PE ~24us incl. warms.  The fixed
compiler epilogue (all-sem clear + barriers) adds ~9.2us inside the
measured window; nothing in-kernel can remove it.

Hard-won scheduling rules (measured on HW):
  - ONE clean HWDGE ring (sync) carries everything; a second ring or
    SWDGE traffic fragments packets and starves transfers.
  - Pool (gpsimd) slab ops are poison: ~1175ns each AND concurrent DVE
    ops slow ~3x (shared SBUF ports).  Keep Pool idle.
  - dma_start dispatches block their engine queue on flow-control sems, so
    the scalar/vector/tensor queues carry no DMA at all.
  - PE p-states: 634ns/512-col stream at MID (1.2GHz), 375ns at FULL
    (2.4GHz).  ~4us gapless matmul activity promotes; gaps <= ~1.2us can
    hold it, but a power/thermal limiter demotes unpredictably — design
    for MID, treat FULL as a bonus.
  - DMA engines re-ramp from ~5GB/s after any queue gap; a sacrificial
    warm transfer does NOT absorb the ramp (measured +8us — don't).
  - Run-to-run HW variance is +/-2.5us (thermal) — compare min-of-8.
"""

import numpy as np
import ml_dtypes

import concourse.bass as bass
import concourse.tile as tile
from concourse import bacc, mybir
from concourse.bass_utils import run_bass_kernel_spmd

BF16 = ml_dtypes.bfloat16

B, D, HEAD, DIM = 16384, 1024, 8, 128
NCORES = 8
ROWS = B // NCORES          # 2048 rows per core
P = 128
BATCH = 512                 # rows per batch (moving free dim of matmuls)
NB = ROWS // BATCH          # 4 batches

CMAX = 8.0                  # |c|_inf gate for the t1 fold
N_WARM = 6                  # PE clock-gate warm-up matmuls: sized so the
                            # warm burst runs seamlessly into batch 0's
                            # matmuls — ~4us of gapless activity promotes
                            # the clock MID->FULL (634->379ns streams);
                            # the p-state is also power/thermal limited,
                            # so promotion is best-effort

_nc_cache = []              # [0]: nc, [1]: unfolded tuple it was built for


def build_nc(unfolded=(1,)):
    nc = bacc.Bacc(target_bir_lowering=False)
    f32 = mybir.dt.float32
    bf16 = mybir.dt.bfloat16
    Act = mybir.ActivationFunctionType

    x1t_d = nc.dram_tensor("x1t", [NB, P, HEAD, BATCH], bf16,
                           kind="ExternalInput")
    x2t_d = nc.dram_tensor("x2t", [NB, P, HEAD, BATCH], bf16,
                           kind="ExternalInput")
    w3t_d = nc.dram_tensor("w3t", [DIM, HEAD, DIM], bf16, kind="ExternalInput")
    w12f_d = nc.dram_tensor("w12f", [DIM, 2, HEAD], f32, kind="ExternalInput")
    out_d = nc.dram_tensor("out", [NB * BATCH], f32, kind="ExternalOutput")

    with tile.TileContext(nc) as tc:
        with (
            tc.tile_pool(name="const", bufs=1) as const_pool,
            tc.tile_pool(name="xt", bufs=NB) as xt_pool,
            tc.tile_pool(name="s", bufs=8) as s_pool,
            tc.tile_pool(name="prod", bufs=3) as prod_pool,
            tc.tile_pool(name="acc", bufs=2) as acc_pool,
            tc.tile_pool(name="res", bufs=1) as res_pool,
            tc.tile_pool(name="yps", bufs=5, space="PSUM") as yps_pool,
            tc.tile_pool(name="rps", bufs=2, space="PSUM") as rps_pool,
        ):
            ones = const_pool.tile([DIM, 1], bf16)
            nc.vector.memset(ones, 1.0)

            # weights ride the FRONT of the sync ring (a separate ring gets
            # starved by the bulk stream; measured 10us for 275KB).
            w12b = const_pool.tile([DIM, 2, HEAD], bf16)
            w12f = const_pool.tile([DIM, 2, HEAD], f32)
            w3l = const_pool.tile([DIM, HEAD, DIM], bf16)


            x1ts = [xt_pool.tile([P, HEAD, BATCH], bf16, tag="x1t",
                                 name=f"x1s{b}") for b in range(NB)]
            x2ts = [xt_pool.tile([P, HEAD, BATCH], bf16, tag="x2t",
                                 name=f"x2s{b}") for b in range(NB)]
            def ld(ts, td, bat, a, b):
                nc.sync.dma_start(out=ts[bat][:, a:b, :],
                                  in_=td[bat, :, a:b, :])

            # x1 rides one FULL batch ahead of x2 (PE consumes only x1):
            # x1(b+1) halves are woven between x2(b) halves, so the PE/ACT
            # chain for batch b+1 is already hot when the DVE finishes
            # batch b — kills the ~2us ACT-wait stall at each batch
            # boundary.  Ring head: x1 front absorbs the DMA spin-up
            # (first packets crawl at ~5GB/s/engine), then w3l/w12f ride
            # hot.  Last batch's x2 tail arrives in eighth-chunks; the
            # dispatches are all queued early so each tail chunk only pays
            # its own transfer+sem, and only one head trails the final
            # packet.
            ld(x1ts, x1t_d, 0, 0, 4)
            nc.sync.dma_start(out=w3l, in_=w3t_d[:])
            nc.sync.dma_start(out=w12f, in_=w12f_d[:])
            nc.vector.tensor_copy(w12b, w12f)
            ld(x2ts, x2t_d, 0, 0, 2)
            ld(x1ts, x1t_d, 0, 4, 8)
            ld(x2ts, x2t_d, 0, 2, 4)
            for bat in range(1, NB):
                ld(x1ts, x1t_d, bat, 0, 4)
                if bat == NB - 1:
                    # tail batch's x1 back half rides 1.5 batches ahead:
                    # at MID clock s(h7) <- ACT <- W3 <- x1[4:8] lands
                    # only ~0.1us before the last x2 chunk — this buys the
                    # tail s-chain 0.64us of margin, delaying only b2's
                    # non-critical finalize.
                    ld(x1ts, x1t_d, bat, 4, 8)
                    ld(x2ts, x2t_d, bat - 1, 4, 8)
                else:
                    ld(x2ts, x2t_d, bat - 1, 4, 8)
                    ld(x1ts, x1t_d, bat, 4, 8)
                ld(x2ts, x2t_d, bat, 0, 4)
            ld(x2ts, x2t_d, NB - 1, 4, 6)
            ld(x2ts, x2t_d, NB - 1, 6, 8)

            # warm the ACT function table while DMA streams
            warm = const_pool.tile([1, 1], f32)
            nc.scalar.activation(warm, ones[0:1, 0:1], Act.Identity,
                                 bias=0.0, scale=1.0)

            # warm the PE HAM clock-gate during the DMA fill
            scratch = const_pool.tile([DIM, BATCH], bf16)
            nc.vector.memset(scratch, 0.0)
            wps = yps_pool.tile([DIM, BATCH], f32, tag="warm", bufs=1)
            for _ in range(N_WARM):
                nc.tensor.matmul(wps, scratch[:, 0:DIM], scratch,
                                 start=True, stop=True)

            res = res_pool.tile([1, NB * BATCH], f32)

            # deferred thunks so the PE never waits on the DVE round
            # trip: finalizers (prev batch reduce+copy+store) drain at h==5
            # of the next batch; the last batch's tail reduces use their own
            # lagged list.
            finalizers = []
            tail = []

            def drain(lst, n):
                while len(lst) > n:
                    lst.pop(0)()

            for bat in range(NB):
                last = bat == NB - 1
                x1t, x2t = x1ts[bat], x2ts[bat]
                rps = rps_pool.tile([1, BATCH], f32)
                prods = prod_pool.tile([DIM, HEAD, BATCH], bf16, tag="p")
                pairs = prod_pool.tile([DIM, HEAD // 2, BATCH], bf16,
                                       tag="q")
                quads = acc_pool.tile([DIM, 2, BATCH], bf16, tag="a")
                state = {"open": False}

                def touch_rps(rps=rps, state=state):
                    st = not state["open"]
                    state["open"] = True
                    return st

                def finalize(rps=rps, quads=quads, bat=bat, state=state):
                    nc.tensor.matmul(rps, ones, quads[:, 0, :],
                                     start=not state["open"], stop=False)
                    nc.tensor.matmul(rps, ones, quads[:, 1, :],
                                     start=False, stop=True)
                    nc.scalar.copy(res[:, bat * BATCH:(bat + 1) * BATCH],
                                   rps)
                    nc.sync.dma_start(
                        out=out_d[bat * BATCH:(bat + 1) * BATCH],
                        in_=res[:, bat * BATCH:(bat + 1) * BATCH])

                s2 = None
                for h in range(HEAD):
                    yps = yps_pool.tile([DIM, BATCH], f32)
                    nc.tensor.matmul(yps, w3l[:, h, :], x1t[:, h, :],
                                     start=True, stop=True)
                    if h in unfolded:
                        nc.tensor.matmul(rps, w12b[:, 0, h:h + 1],
                                         x1t[:, h, :],
                                         start=touch_rps(), stop=False)
                    if h % 2 == 0:
                        s2 = s_pool.tile([DIM, 2, BATCH], bf16, tag="s")
                    nc.scalar.activation(s2[:, h % 2, :], yps, Act.Identity,
                                         bias=w12f[:, 1, h:h + 1],
                                         scale=1.0)
                    if h == 7:
                        # drain AFTER all of this batch's W3 matmuls AND
                        # after the h7 activation: at h==5 the prev batch's
                        # reduce streams displace W3 h6/h7 by ~1.2us, and
                        # draining before the ACT emission would park the
                        # prev batch's res-copy ahead of s(h7) in the
                        # in-order ACT queue — blocking the tail-critical
                        # s on the last batch.
                        drain(finalizers, 0)
                    if not last:
                        # 2-head fused mul + tree sum as wide contiguous
                        # DVE ops — one op amortizes the ~175ns fixed
                        # overhead over 1024/2048 elements.  Pool is
                        # avoided entirely: its slab ops measured 1175ns
                        # AND slow concurrent DVE ops ~3x (shared SBUF
                        # ports).
                        if h % 2 == 1:
                            nc.vector.tensor_mul(prods[:, h - 1:h + 1, :],
                                                 s2, x2t[:, h - 1:h + 1, :])
                        if h == 7:
                            nc.vector.tensor_add(pairs,
                                                 prods[:, 0:4, :],
                                                 prods[:, 4:8, :])
                            nc.vector.tensor_add(quads,
                                                 pairs[:, 0:2, :],
                                                 pairs[:, 2:4, :])
                    else:
                        # tail batch: per-head muls; heads 0-5 pair-add on
                        # the DVE (their chunks land mid-stream, filling
                        # DVE idle) so the serial PE reduce train is 5
                        # streams, not 8 — the train itself is the tail
                        # pacer at MID clock (634ns each, inputs ready).
                        # Heads 6/7 reduce directly (shortest chain after
                        # the last packet).
                        prod = prods[:, h, :]
                        nc.vector.tensor_mul(prod, s2[:, h % 2, :],
                                             x2t[:, h, :])
                        if h in (1, 3, 5):
                            prod = pairs[:, h // 2, :]
                            nc.vector.tensor_add(prod, prods[:, h - 1, :],
                                                 prods[:, h, :])
                        elif h < 6:
                            continue
                        def red(rps=rps, prod=prod, h=h, bat=bat):
                            nc.tensor.matmul(rps, ones, prod,
                                             start=touch_rps(),
                                             stop=(h == HEAD - 1))
                            if h == HEAD - 1:
                                nc.scalar.copy(
                                    res[:, bat * BATCH:(bat + 1) * BATCH],
                                    rps)
                                nc.sync.dma_start(
                                    out=out_d[bat * BATCH:
                                              (bat + 1) * BATCH],
                                    in_=res[:, bat * BATCH:
                                            (bat + 1) * BATCH])
                        # do NOT drain mid-loop: a red waits its x2-gated
                        # prod, and parking it in the in-order PE queue
                        # before W3(h+1..) blocks the x1-ready matmuls —
                        # delaying the tail-critical s(h6,h7) chain.  All
                        # reds drain after the loop, behind W3(h7)/s(h7).
                        tail.append(red)
                if not last:
                    finalizers.append(finalize)
                else:
                    drain(tail, 0)

    nc.finalize()
    return nc


def _fold(W1, W2, W3):
    """Per-head c_h with |c|_inf <= CMAX s.t. W3_h^T c_h = W1_h (t1 fold).

    Returns (cs [HEAD, DIM] float64, unfolded tuple, const): heads whose
    solve is too ill-conditioned keep c=0 and an explicit t1 matmul.
    """
    W1 = np.asarray(W1, np.float64)
    W2 = np.asarray(W2, np.float64)
    w3b = np.asarray(W3, np.float32).astype(BF16).astype(np.float64)
    cs = np.zeros((HEAD, DIM))
    unfolded = []
    for h in range(HEAD):
        try:
            c = np.linalg.solve(w3b[h].T, W1[h])
        except np.linalg.LinAlgError:
            c = np.full(DIM, np.inf)
        if np.abs(c).max() <= CMAX and np.all(np.isfinite(c)):
            cs[h] = c
        else:
            unfolded.append(h)
    const = -(W2 * cs).sum()
    return cs, tuple(unfolded), const


def _prep_weights(W1, W2, W3):
    # W3 is [h, o, i]; lhsT needs [i (partitions), h, o]
    w3t = np.ascontiguousarray(
        np.transpose(np.asarray(W3), (2, 0, 1))).astype(BF16)
    w12f = np.empty((DIM, 2, HEAD), dtype=np.float32)
    w12f[:, 0, :] = np.asarray(W1).T   # [i, h]
    w12f[:, 1, :] = np.asarray(W2).T   # [o, h]
    return w3t, w12f


def _prep_x(x, cs=None):
    """[B, D] fp32 -> per-core [NB, P, HEAD, BATCH] bf16, pre-transposed.
    cs: optional [HEAD, DIM] fold offsets added before the bf16 cast."""
    xv = np.asarray(x, dtype=np.float64).reshape(NCORES, NB, BATCH, HEAD, DIM)
    if cs is not None:
        xv = xv + cs[None, None, None]
    xb = xv.astype(BF16)
    # [core, bat, b, h, i] -> [core, bat, i, h, b]
    v = xb.transpose(0, 1, 4, 3, 2)
    return np.ascontiguousarray(v)


def _in_maps(x1, x2, W1, W2, W3):
    cs, unfolded, const = _fold(W1, W2, W3)
    w3t, w12f = _prep_weights(W1, W2, W3)
    x1t = _prep_x(x1)
    x2t = _prep_x(x2, cs)
    maps = [
        {"x1t": x1t[c], "x2t": x2t[c], "w3t": w3t, "w12f": w12f}
        for c in range(NCORES)
    ]
    return maps, unfolded, const


def kernel(x1, x2, W1, b1, W2, W3):
    in_maps, unfolded, const = _in_maps(x1, x2, W1, W2, W3)
    if not _nc_cache or _nc_cache[1] != unfolded:
        _nc_cache.clear()
        _nc_cache.append(build_nc(unfolded))
        _nc_cache.append(unfolded)
    nc = _nc_cache[0]

    c_all = float(np.asarray(b1, dtype=np.float64).sum() + const)

    res = run_bass_kernel_spmd(nc, in_maps, core_ids=list(range(NCORES)))
    out = np.concatenate(
        [res.results[c]["out"].reshape(-1) for c in range(NCORES)])
    return (out + np.float32(c_all)).astype(np.float32)
